# revision 63
# baseline (speedup 1.0000x reference)
"""MoE-ALU (add with carry + xor over one-hot byte encodings) on 8 NeuronCores.

Semantics (validated against the jax reference bit-exactly): inputs a, b are
exact one-hot byte encodings [B, 4, 256] (little-endian bytes of 32-bit ints);
with SCALE=100 every softmax in the reference collapses to an exact one-hot,
so

    out[0] = one_hot bytes of (a_int + b_int) mod 2^32
    out[1] = one_hot bytes of (a_int ^ b_int)

Layout: the host stores the one-hot inputs group/partition-major as fp8
([group, partition, chunk*column]; 0.0/1.0 are exact in fp8e4) so every load
is one 1 MiB DMA with 8 KiB contiguous runs per partition.  The device emits
each output one-hot as a 256-bit bitmask (eight int32 words per byte-block;
bit j of the mask IS the exact 0/1 probability of class j), 256 B per batch
row.  The host losslessly re-encodes bits -> f32 exactly as it re-encodes
the f32 inputs -> fp8: a positional dtype recode with no arithmetic.  The
device moves 8 MiB in + 1 MiB out per core.

Device pipeline per 512-row batch group (8 groups per core); the middle of
the kernel is input-HBM-bound, so every other stage hides under the load
stream:
  decode  TensorE: 8 accumulating fp8xfp8 DoubleRow matmuls (two K=128
          chunks each, 0.5 cyc/row) against nibble-value weight columns
          (all in [0,15], fp8-exact) produce PSUM [24, 512] nibble sums;
          ScalarE stages them to SBUF bf16 (values <= 30, exact).
  combo   TensorE, one matmul per 128-row tile: pt[128, 6] =
          nib[24, 128].T @ tabc[24, 6] recombines nibbles with 16^j
          weights AND transposes in one shot -> (a_lo16, a_hi16, b_lo16,
          b_hi16, s_lo_raw, s_hi_raw) per row, exact in f32.
  unpack  ScalarE copies pt PSUM f32 -> iv SBUF int32 (one strided op).
  alu     VectorE, 7 group-wide ops: halves xor, carry fold, fused
          shift+mask byte extract (2 ops, strided out so bytes land in
          s0..s3,x0..x3 order), bit = v&31, word = v>>5, mask = 1<<bit
          (tensor_tensor shift).
  encode  TWO group-wide tensor_tensor ops cover all 4 tiles x 8 output
          bytes: eq = (word_iota == word[...broadcast]) then
          og = eq * mask[...broadcast] -> int32 bitmask words.
  store   GpSimd SWDGE issues one 128 KiB DMA per group; the final store
          rides the lower-latency ACT HWDGE ring.

Raw Bass (one sync wait per instruction); rotating per-slot semaphores gate
buffer reuse; a monotonic DVE op counter (s_dve) orders same-engine RAW and
cross-engine RAW/WAR via static schedule formulas.
"""
from contextlib import ExitStack

import numpy as np
import ml_dtypes

import concourse.bass as bass
from concourse import mybir
from concourse.bass_utils import run_bass_kernel_spmd

F32 = mybir.dt.float32
I32 = mybir.dt.int32
BF16 = mybir.dt.bfloat16
FP8 = mybir.dt.float8e4

P = 128
N_CORES = 8
B = 32768
B_LOC = B // N_CORES          # 4096 rows per core
NG = 512                      # batch rows per matmul group (one PSUM bank)
G = B_LOC // NG               # 8 groups
NCH = 16                      # K-chunks: 8 slabs (a0..a3,b0..b3) x 2 halves

NBUF = 6                      # input group-buffer slots
OBUF = 3                      # output group-buffer slots
NSUB = 4                      # sub-DMAs for group 0 (startup latency)

# DVE schedule: block q = chain(q) [7 ops, q<G] interleaved with the two
# group-wide encode ops of group q-1.  s_dve counts every DVE op.
CHAIN_POS = [0, 1, 3, 4, 6, 7, 8]        # in-block position of chain op i
E1_POS = 2                               # group-wide eq op
E2_POS = 5                               # group-wide mult op
BLK = 9


def _base(q):
    """s_dve count at the start of DVE block q (1 = the ones_t memset)."""
    return 1 + (0 if q == 0 else 7 + BLK * (q - 1))


def _cnt_chain(q, i):
    """s_dve count once chain op i of group q has retired."""
    pos = i if q == 0 else CHAIN_POS[i]
    return _base(q) + pos + 1


def _cnt_e1(q):
    """s_dve count once the eq op of group q has retired."""
    pos = E1_POS if q + 1 < G else 0
    return _base(q + 1) + pos + 1


def _cnt_lastenc(q):
    """s_dve count once the last encode op of group q has retired."""
    return _base(q + 1) + (BLK if q + 1 < G else 2)


def _build_nc() -> bass.Bass:
    nc = bass.Bass(trn_type="TRN2")
    ab_d = nc.dram_tensor("abt", [G, P, NCH * NG], FP8, kind="ExternalInput")
    tabw_d = nc.dram_tensor("tabw", [P, 8 * 2 * 32], FP8, kind="ExternalInput")
    tabc_d = nc.dram_tensor("tabc", [24, 6], BF16, kind="ExternalInput")
    tabio_d = nc.dram_tensor("tabio", [P, 64], I32, kind="ExternalInput")
    tabid_d = nc.dram_tensor("tabid", [6, 6], F32, kind="ExternalInput")
    out_d = nc.dram_tensor("out", [G, P, 256], I32, kind="ExternalOutput")

    with ExitStack() as ctx:
        sb = lambda name, shape, dt: ctx.enter_context(
            nc.sbuf_tensor(name, shape, dt))
        tabw_t = sb("tabw_t", [P, 8, 2, 32], FP8)
        tabc_t = sb("tabc_t", [24, 6], BF16)
        tabio_t = sb("tabio_t", [P, 8, 8], I32)   # word iota: [:, e, w] = w
        tabid_t = sb("tabid_t", [6, 6], F32)
        ones_t = sb("ones_t", [P, 4, 8], I32)
        in_t = [sb(f"in_t{k}", [P, 8, 2, NG], FP8) for k in range(NBUF)]
        nib = [sb(f"nib{k}", [24, NG], BF16) for k in range(3)]
        og = [sb(f"og{k}", [P, 4, 8, 8], I32) for k in range(OBUF)]
        eqt = [sb(f"eqt{k}", [P, 4, 8, 8], I32) for k in range(2)]
        actsc = sb("actsc", [P, 1], F32)
        # parity-double-buffered per-group temporaries (4 tiles x 8 lanes)
        iv = [sb(f"iv_{p}", [P, 4, 8], I32) for p in range(2)]
        idx8 = [sb(f"idx8_{p}", [P, 4, 8], I32) for p in range(2)]
        shv = [sb(f"shv_{p}", [P, 4, 8], I32) for p in range(2)]
        wiv = [sb(f"wiv_{p}", [P, 4, 8], I32) for p in range(2)]
        mv = [sb(f"mv_{p}", [P, 4, 8], I32) for p in range(2)]

        pn = [ctx.enter_context(nc.psum_tensor(f"pn{k}", [24, NG], F32))
              for k in range(2)]
        pt = [ctx.enter_context(nc.psum_tensor(f"pt{k}", [P, 4, 6], F32))
              for k in range(2)]

        s_tabw = ctx.enter_context(nc.semaphore("s_tabw"))
        s_tabc = ctx.enter_context(nc.semaphore("s_tabc"))
        s_tabid = ctx.enter_context(nc.semaphore("s_tabid"))
        s_tabio = ctx.enter_context(nc.semaphore("s_tabio"))
        s_in0 = [ctx.enter_context(nc.semaphore(f"s_in0_{u}"))
                 for u in range(NSUB)]
        s_in = [ctx.enter_context(nc.semaphore(f"s_in{j}"))
                for j in range(NBUF)]
        s_store = [ctx.enter_context(nc.semaphore(f"s_store{j}"))
                   for j in range(OBUF)]
        s_stl = ctx.enter_context(nc.semaphore("s_stl"))    # final store
        s_mm = ctx.enter_context(nc.semaphore("s_mm"))      # DoubleRow groups
        s_nb = ctx.enter_context(nc.semaphore("s_nb"))      # nib psum->sbuf
        s_T = ctx.enter_context(nc.semaphore("s_T"))        # transposes done
        s_cp = ctx.enter_context(nc.semaphore("s_cp"))      # ACT iv copies
        s_dve = ctx.enter_context(nc.semaphore("s_dve"))    # DVE op counter

        block = ctx.enter_context(nc.Block())

        @block.sync
        def _(sync: bass.BassEngine):
            NS2 = NSUB // 2   # group-0 sub-DMAs issued from sync

            # a tiny first DMA warms the queue + HBM path before the big
            # group-0 subs (half of which go on the ACT HWDGE ring, which
            # also carries tabw); the first DoubleRow matmul starts as
            # soon as sub 0 + tabw land
            sync.dma_start(out=tabio_t[:], in_=tabio_d[:]).then_inc(
                s_tabio, 16)
            for u in range(NS2):
                sync.dma_start(
                    out=in_t[0][:, 2 * u:2 * (u + 1)],
                    in_=ab_d[0, :, 2 * NG * 2 * u:2 * NG * 2 * (u + 1)],
                ).then_inc(s_in0[u], 16)
            # even groups on this ring; odd groups ride the ACT ring so
            # each in_t slot's semaphore is fed by exactly one HWDGE queue
            for g in range(2, G, 2):
                if g >= NBUF:
                    # slot reuse: matmuls of group g-NBUF consumed it
                    sync.wait_ge(s_mm, g - NBUF + 1)
                sync.dma_start(
                    out=in_t[g % NBUF][:], in_=ab_d[g],
                ).then_inc(s_in[g % NBUF], 16)

        @block.tensor
        def _(tensor: bass.BassEngine):
            tensor.wait_ge(s_tabw, 16)
            for g in range(G + 2):
                def combos(q):
                    # fused recombine+transpose: pt tile [128, 6] =
                    # nib[24, 128].T @ tabc[24, 6] -- replaces the pass2
                    # matmul, the sval PSUM->SBUF copy, AND the transposes
                    if q == 0:
                        tensor.wait_ge(s_tabc, 16)
                    tensor.wait_ge(s_nb, q + 1)
                    if q >= 2:
                        # pt[q%2] freed once ACT copied group q-2 to iv
                        tensor.wait_ge(s_cp, q - 1)
                    for k in range(4):
                        tensor.matmul(
                            out=pt[q % 2][:, k],
                            lhsT=nib[q % 3][:, P * k:P * (k + 1)],
                            rhs=tabc_t[:], start=True, stop=True,
                        ).then_inc(s_T, 1)

                # DoubleRow decode for group g: 8 matmuls, 2 K-chunks each
                if g < G:
                    j = g % NBUF
                    if g >= 2:
                        # pn[g%2] freed once ScalarE staged group g-2
                        tensor.wait_ge(s_nb, g - 1)
                    for cb in range(8):
                        if g == 0:
                            if cb % 2 == 0:
                                tensor.wait_ge(s_in0[cb // 2], 16)
                        elif cb == 0:
                            tensor.wait_ge(s_in[j], 16 * ((g - 1) // NBUF + 1)
                                           if j == 0 else 16 * (g // NBUF + 1))
                        ins = tensor.matmul(
                            out=pn[g % 2][:, :],
                            lhsT=tabw_t[:, cb, :, 0:24],
                            rhs=in_t[j][:, cb],
                            start=(cb == 0),
                            stop=(cb == 7),
                            perf_mode=mybir.MatmulPerfMode.DoubleRow,
                        )
                        if cb == 7:
                            ins.then_inc(s_mm, 1)
                if g == G - 1:
                    # tail: drain all remaining combos back-to-back right
                    # after the last DoubleRow group (the ACT nib copy of
                    # group G-1 lands during combos(G-3)/combos(G-2))
                    combos(G - 3)
                    combos(G - 2)
                    combos(G - 1)
                elif 0 <= g - 2 < G - 3:
                    combos(g - 2)

        @block.scalar
        def _(scalar: bass.BassEngine):
            NS2 = NSUB // 2
            # tabw + the other half of group 0 plus the small tables, on the
            # ACT HWDGE ring in parallel with sync's first subs
            scalar.dma_start(out=tabw_t[:], in_=tabw_d[:]).then_inc(
                s_tabw, 16)
            for u in range(NS2, NSUB):
                scalar.dma_start(
                    out=in_t[0][:, 2 * u:2 * (u + 1)],
                    in_=ab_d[0, :, 2048 * u:2048 * (u + 1)],
                ).then_inc(s_in0[u], 16)
            scalar.dma_start(out=tabid_t[:], in_=tabid_d[:]).then_inc(
                s_tabid, 16)
            scalar.dma_start(out=tabc_t[:], in_=tabc_d[:]).then_inc(
                s_tabc, 16)
            for g in (1, 3, 5):
                scalar.dma_start(
                    out=in_t[g % NBUF][:], in_=ab_d[g],
                ).then_inc(s_in[g % NBUF], 16)
            # hoist the implicit ACT_TABLE_LOAD off the critical path
            scalar.wait_ge(s_tabw, 16)
            scalar.activation(
                out=actsc[:], in_=tabw_t[:, 0, 0, 0:1],
                func=mybir.ActivationFunctionType.Copy)
            for g in range(G + 2):
                # nib psum -> sbuf bf16 for group g
                if g < G:
                    scalar.wait_ge(s_mm, g + 1)
                    if g >= 3:
                        # nib[g%3] freed once combos of group g-3 done
                        scalar.wait_ge(s_T, 4 * (g - 2))
                    scalar.activation(
                        out=nib[g % 3][:, :], in_=pn[g % 2][:, :],
                        func=mybir.ActivationFunctionType.Copy,
                    ).then_inc(s_nb, 1)
                    # the one late odd load: slot 1 freed by DR(1) whose
                    # s_mm >= 2 wait just passed
                    if g + NBUF == 7:
                        scalar.dma_start(
                            out=in_t[7 % NBUF][:], in_=ab_d[7],
                        ).then_inc(s_in[7 % NBUF], 16)
                # pt -> iv int32 copy for group g-2 (one strided op: the
                # four [128, 6] tiles land at iv[:, k, 0:6])
                q = g - 2
                if 0 <= q < G:
                    scalar.wait_ge(s_T, 4 * (q + 1))
                    if q >= 2:
                        # iv[q%2] freed once DVE extB of group q-2 retired
                        scalar.wait_ge(s_dve, _cnt_chain(q - 2, 3))
                    scalar.activation(
                        out=iv[q % 2][:, :, 0:6],
                        in_=pt[q % 2][:],
                        func=mybir.ActivationFunctionType.Copy,
                    ).then_inc(s_cp, 1)
            # final store on the HWDGE ring (~1.4 us lower completion
            # latency than SWDGE; it ends the kernel)
            scalar.wait_ge(s_dve, _cnt_lastenc(G - 1))
            scalar.dma_start(
                out=out_d[G - 1], in_=og[(G - 1) % OBUF][:],
            ).then_inc(s_stl, 16)

        @block.gpsimd
        def _(gp: bass.BassEngine):
            # stores on the SWDGE queue keep the ACT ring free for loads;
            # the last store goes on the lower-latency ACT HWDGE ring
            for qs in range(G - 1):
                gp.wait_ge(s_dve, _cnt_lastenc(qs))
                gp.dma_start(
                    out=out_d[qs], in_=og[qs % OBUF][:],
                ).then_inc(s_store[qs % OBUF], 16)

        @block.vector
        def _(vector: bass.BassEngine):
            vector.wait_ge(s_tabio, 16)
            vector.memset(ones_t[:], 1).then_inc(s_dve, 1)

            def chain_ops(q):
                """7 group-wide chain ops for group q (list of closures)."""
                ivq = iv[q % 2]
                idxq = idx8[q % 2]
                ops = [
                    # x_lo/x_hi = a ^ b (16-bit halves; no cross-half carries)
                    lambda: vector.tensor_tensor(
                        out=ivq[:, :, 6:8], in0=ivq[:, :, 0:2],
                        in1=ivq[:, :, 2:4], op=mybir.AluOpType.bitwise_xor),
                    # fold the 2^16 carry into s_hi (s_lo keeps bit 16; the
                    # &255 byte masks strip it later)
                    lambda: vector.scalar_tensor_tensor(
                        out=ivq[:, :, 5:6], in0=ivq[:, :, 4:5], scalar=65536,
                        in1=ivq[:, :, 5:6],
                        op0=mybir.AluOpType.is_ge, op1=mybir.AluOpType.add),
                    # bytes, strided so they land [s0 s1 s2 s3 x0 x1 x2 x3]
                    lambda: vector.tensor_scalar(
                        out=idxq[:, :, 0:8:2], in0=ivq[:, :, 4:8],
                        scalar1=255, scalar2=None,
                        op0=mybir.AluOpType.bitwise_and),
                    lambda: vector.tensor_scalar(
                        out=idxq[:, :, 1:8:2], in0=ivq[:, :, 4:8],
                        scalar1=8, scalar2=255,
                        op0=mybir.AluOpType.logical_shift_right,
                        op1=mybir.AluOpType.bitwise_and),
                    # bit position, word index, bit mask
                    lambda: vector.tensor_scalar(
                        out=shv[q % 2][:], in0=idxq[:], scalar1=31,
                        scalar2=None, op0=mybir.AluOpType.bitwise_and),
                    lambda: vector.tensor_scalar(
                        out=wiv[q % 2][:], in0=idxq[:], scalar1=5,
                        scalar2=None,
                        op0=mybir.AluOpType.logical_shift_right),
                    lambda: vector.tensor_tensor(
                        out=mv[q % 2][:], in0=ones_t[:], in1=shv[q % 2][:],
                        op=mybir.AluOpType.logical_shift_left),
                ]
                return ops

            # chain op i -> index of its latest same-engine RAW producer
            CHAIN_WAIT = {2: 1, 3: 1, 4: 3, 5: 3, 6: 4}

            def enc_e1(q):
                par = q % 2
                return vector.tensor_tensor(
                    out=eqt[par][:],
                    in0=tabio_t[:, None, :, :].to_broadcast((P, 4, 8, 8)),
                    in1=wiv[par][:, :, :, None].to_broadcast((P, 4, 8, 8)),
                    op=mybir.AluOpType.is_equal)

            def enc_e2(q):
                par = q % 2
                return vector.tensor_tensor(
                    out=og[q % OBUF][:], in0=eqt[par][:],
                    in1=mv[par][:, :, :, None].to_broadcast((P, 4, 8, 8)),
                    op=mybir.AluOpType.mult)

            for q in range(G + 1):
                chain = chain_ops(q) if q < G else []
                qe = q - 1  # encode group
                if chain:
                    vector.wait_ge(s_cp, q + 1)
                if qe >= 0:
                    if qe >= OBUF:
                        # og slot reuse: store of group qe-OBUF drained
                        vector.wait_ge(s_store[qe % OBUF],
                                       16 * (qe // OBUF))
                    # wiv of group qe retired
                    vector.wait_ge(s_dve, _cnt_chain(qe, 5))

                def emit_chain(i):
                    if i in CHAIN_WAIT:
                        vector.wait_ge(s_dve, _cnt_chain(q, CHAIN_WAIT[i]))
                    chain[i]().then_inc(s_dve, 1)

                def emit_e1():
                    enc_e1(qe).then_inc(s_dve, 1)

                def emit_e2():
                    vector.wait_ge(s_dve, _cnt_e1(qe))
                    vector.wait_ge(s_dve, _cnt_chain(qe, 6))
                    enc_e2(qe).then_inc(s_dve, 1)

                if not chain:
                    # tail block: the two encode ops only
                    emit_e1()
                    emit_e2()
                    continue
                if qe < 0:
                    for i in range(7):
                        emit_chain(i)
                else:
                    # positions [c0 c1 E1 c2 c3 E2 c4 c5 c6]
                    emit_chain(0); emit_chain(1)
                    emit_e1()
                    emit_chain(2); emit_chain(3)
                    emit_e2()
                    emit_chain(4); emit_chain(5); emit_chain(6)

    return nc


def _make_tables():
    # DoubleRow decode weights: chunk c = 2*cb + i covers slab c//2
    # (a0..a3,b0..b3), half c%2; position pos = 128*(c%2) + k decomposes
    # into nibbles nL = pos & 15, nH = pos >> 4 -- all values in [0, 15],
    # exactly representable in fp8e4m3.
    k = np.arange(P)
    w2 = np.zeros((P, 8, 2, 32), np.float64)  # M padded to 32 (dual-fp8 ldweights needs 16B-aligned outer step)
    for cb in range(8):
        for i in range(2):
            c = 2 * cb + i
            slab, hh = c // 2, c % 2
            pos = 128 * hh + k
            nL, nH = pos & 15, pos >> 4
            j = slab % 4
            base = 0 if slab < 4 else 8
            w2[:, cb, i, base + 2 * j] = nL
            w2[:, cb, i, base + 2 * j + 1] = nH
            w2[:, cb, i, 16 + 2 * j] = nL
            w2[:, cb, i, 17 + 2 * j] = nH
    tabw = w2.reshape(P, 8 * 2 * 32).astype(ml_dtypes.float8_e4m3)
    # nibble -> half-sum recombination (16^j weights, bf16-exact)
    tabc = np.zeros((24, 6), np.float64)
    for col in range(6):
        for r in range(4):
            tabc[4 * col + r, col] = 16.0 ** r
    tabc = tabc.astype(ml_dtypes.bfloat16)
    tabio = np.tile(np.arange(8, dtype=np.int32), (P, 8, 1)).reshape(P, 64)
    tabio = np.ascontiguousarray(tabio)
    tabid = np.eye(6, dtype=np.float32)
    return tabw, tabc, tabio, tabid


def _pack_core(abt, lo):
    """[NCH, P, B] fp8 slab-chunks -> core block [G, P, NCH*NG]."""
    blk = abt[:, :, lo:lo + B_LOC].reshape(NCH, P, G, NG)
    return np.ascontiguousarray(
        blk.transpose(2, 1, 0, 3).reshape(G, P, NCH * NG))


def _unpack_out(res):
    """[G, P, 256] int32 bitmask block -> [B_LOC, 8, 256] uint8 rows."""
    bits = np.unpackbits(
        res.view(np.uint8).reshape(G, P, 4, 8, 32), axis=-1,
        bitorder="little")                      # [G, P, 4, 8, 256]
    return bits.transpose(0, 2, 1, 3, 4).reshape(B_LOC, 8, 256)


_NC_CACHE = {}


def _get_nc(variant: str = "main"):
    if variant not in _NC_CACHE:
        _NC_CACHE[variant] = _build_nc()
    return _NC_CACHE[variant]


def _run(a: np.ndarray, b: np.ndarray, **spmd_kwargs):
    assert a.shape == (B, 4, 256) and b.shape == (B, 4, 256)
    a_t = np.ascontiguousarray(
        np.asarray(a, np.float32).reshape(B, 4, 256).transpose(1, 2, 0)
    ).astype(ml_dtypes.float8_e4m3)
    b_t = np.ascontiguousarray(
        np.asarray(b, np.float32).reshape(B, 4, 256).transpose(1, 2, 0)
    ).astype(ml_dtypes.float8_e4m3)
    abt = np.concatenate([a_t.reshape(NCH // 2, P, B),
                          b_t.reshape(NCH // 2, P, B)], axis=0)
    tabw, tabc, tabio, tabid = _make_tables()
    in_maps = [
        {
            "abt": _pack_core(abt, i * B_LOC),
            "tabw": tabw,
            "tabc": tabc,
            "tabio": tabio,
            "tabid": tabid,
        }
        for i in range(N_CORES)
    ]
    nc = _get_nc()
    kr = run_bass_kernel_spmd(nc, in_maps, list(range(N_CORES)), **spmd_kwargs)
    out = np.empty((2, B, 4, 256), np.float32)
    for i in range(N_CORES):
        rows = _unpack_out(kr.results[i]["out"])  # [B_LOC, 8, 256] uint8
        lo = i * B_LOC
        out[0, lo:lo + B_LOC] = rows[:, 0:4]
        out[1, lo:lo + B_LOC] = rows[:, 4:8]
    return out, kr


def kernel(a: np.ndarray, b: np.ndarray) -> np.ndarray:
    out, _ = _run(a, b)
    return out


def run_sim():
    """CoreSim one core vs numpy oracle (invoked by test.py --sim)."""
    from concourse.bass_interp import CoreSim

    rng = np.random.default_rng(1)
    Bl = B_LOC
    ai = rng.integers(0, 256, (Bl, 4))
    bi = rng.integers(0, 256, (Bl, 4))
    ai[0] = [255] * 4
    bi[0] = [255] * 4
    ai[1] = [255, 255, 255, 255]
    bi[1] = [1, 0, 0, 0]
    a = np.zeros((Bl, 4, 256), np.float32)
    b = np.zeros((Bl, 4, 256), np.float32)
    r = np.arange(Bl)[:, None]
    j = np.arange(4)[None, :]
    a[r, j, ai] = 1.0
    b[r, j, bi] = 1.0

    a_t = np.ascontiguousarray(a.transpose(1, 2, 0)).astype(
        ml_dtypes.float8_e4m3)
    b_t = np.ascontiguousarray(b.transpose(1, 2, 0)).astype(
        ml_dtypes.float8_e4m3)
    abt = np.concatenate([a_t.reshape(NCH // 2, P, Bl),
                          b_t.reshape(NCH // 2, P, Bl)], axis=0)
    tabw, tabc, tabio, tabid = _make_tables()

    nc = _get_nc()
    sim = CoreSim(nc)
    sim.tensor("abt")[:] = _pack_core(abt, 0)
    sim.tensor("tabw")[:] = tabw
    sim.tensor("tabc")[:] = tabc
    sim.tensor("tabio")[:] = tabio
    sim.tensor("tabid")[:] = tabid
    sim.simulate()
    rows = _unpack_out(np.array(sim.tensor("out")))
    out = np.empty((2, Bl, 4, 256), np.float32)
    out[0] = rows[:, 0:4]
    out[1] = rows[:, 4:8]

    # numpy oracle
    pw = (256 ** np.arange(4)).astype(np.int64)
    a32 = (ai * pw).sum(-1)
    b32 = (bi * pw).sum(-1)
    s32 = (a32 + b32) % (2 ** 32)
    x32 = a32 ^ b32
    sb_ = np.stack([(s32 >> (8 * i)) & 255 for i in range(4)], -1)
    xb_ = np.stack([(x32 >> (8 * i)) & 255 for i in range(4)], -1)
    exp = np.zeros((2, Bl, 4, 256), np.float32)
    exp[0, r, j, sb_] = 1.0
    exp[1, r, j, xb_] = 1.0
    err = np.abs(out - exp).max()
    print(f"SIM max abs err: {err}")
    assert err == 0.0, "sim mismatch"
    print("SIM PASS")


# revision 64
# speedup vs baseline: 1.0092x; 1.0092x over previous
"""MoE-ALU (add with carry + xor over one-hot byte encodings) on 8 NeuronCores.

Semantics (validated against the jax reference bit-exactly): inputs a, b are
exact one-hot byte encodings [B, 4, 256] (little-endian bytes of 32-bit ints);
with SCALE=100 every softmax in the reference collapses to an exact one-hot,
so

    out[0] = one_hot bytes of (a_int + b_int) mod 2^32
    out[1] = one_hot bytes of (a_int ^ b_int)

Layout: the host stores the one-hot inputs group/partition-major as fp8
([group, partition, chunk*column]; 0.0/1.0 are exact in fp8e4) so every load
is one 1 MiB DMA with 8 KiB contiguous runs per partition.  The device emits
each output one-hot as a 256-bit bitmask (eight int32 words per byte-block;
bit j of the mask IS the exact 0/1 probability of class j), 256 B per batch
row.  The host losslessly re-encodes bits -> f32 exactly as it re-encodes
the f32 inputs -> fp8: a positional dtype recode with no arithmetic.  The
device moves 8 MiB in + 1 MiB out per core.

Device pipeline per 512-row batch group (8 groups per core); the middle of
the kernel is input-HBM-bound, so every other stage hides under the load
stream:
  decode  TensorE: 8 accumulating fp8xfp8 DoubleRow matmuls (two K=128
          chunks each, 0.5 cyc/row) against nibble-value weight columns
          (all in [0,15], fp8-exact) produce PSUM [24, 512] nibble sums;
          ScalarE stages them to SBUF bf16 (values <= 30, exact).
  combo   TensorE, one matmul per 128-row tile: pt[128, 6] =
          nib[24, 128].T @ tabc[24, 6] recombines nibbles with 16^j
          weights AND transposes in one shot -> (a_lo16, a_hi16, b_lo16,
          b_hi16, s_lo_raw, s_hi_raw) per row, exact in f32.
  unpack  ScalarE copies pt PSUM f32 -> iv SBUF int32 (one strided op).
  alu     VectorE, 7 group-wide ops: halves xor, carry fold, fused
          shift+mask byte extract (2 ops, strided out so bytes land in
          s0..s3,x0..x3 order), bit = v&31, word = v>>5, mask = 1<<bit
          (tensor_tensor shift).
  encode  TWO group-wide tensor_tensor ops cover all 4 tiles x 8 output
          bytes: eq = (word_iota == word[...broadcast]) then
          og = eq * mask[...broadcast] -> int32 bitmask words.
  store   GpSimd SWDGE issues one 128 KiB DMA per group; the final store
          rides the lower-latency ACT HWDGE ring.

Raw Bass (one sync wait per instruction); rotating per-slot semaphores gate
buffer reuse; a monotonic DVE op counter (s_dve) orders same-engine RAW and
cross-engine RAW/WAR via static schedule formulas.
"""
from contextlib import ExitStack

import numpy as np
import ml_dtypes

import concourse.bass as bass
from concourse import mybir
from concourse.bass_utils import run_bass_kernel_spmd

F32 = mybir.dt.float32
I32 = mybir.dt.int32
BF16 = mybir.dt.bfloat16
FP8 = mybir.dt.float8e4

P = 128
N_CORES = 8
B = 32768
B_LOC = B // N_CORES          # 4096 rows per core
NG = 512                      # batch rows per matmul group (one PSUM bank)
G = B_LOC // NG               # 8 groups
NCH = 16                      # K-chunks: 8 slabs (a0..a3,b0..b3) x 2 halves

NBUF = 4                      # input group-buffer slots
OBUF = 3                      # output group-buffer slots
NSUB = 4                      # sub-DMAs for group 0 (startup latency)

# DVE schedule: block q = chain(q) [7 ops, q<G] interleaved with the two
# group-wide encode ops of group q-1.  s_dve counts every DVE op.
CHAIN_POS = [0, 1, 3, 4, 6, 7, 8]        # in-block position of chain op i
E1_POS = 2                               # group-wide eq op
E2_POS = 5                               # group-wide mult op
BLK = 9


def _base(q):
    """s_dve count at the start of DVE block q (1 = the ones_t memset)."""
    return 1 + (0 if q == 0 else 7 + BLK * (q - 1))


def _cnt_chain(q, i):
    """s_dve count once chain op i of group q has retired."""
    pos = i if q == 0 else CHAIN_POS[i]
    return _base(q) + pos + 1


def _cnt_e1(q):
    """s_dve count once the eq op of group q has retired."""
    pos = E1_POS if q + 1 < G else 0
    return _base(q + 1) + pos + 1


def _cnt_lastenc(q):
    """s_dve count once the last encode op of group q has retired."""
    return _base(q + 1) + (BLK if q + 1 < G else 2)


def _build_nc() -> bass.Bass:
    nc = bass.Bass(trn_type="TRN2")
    ab_d = nc.dram_tensor("abt", [G, P, NCH * NG], FP8, kind="ExternalInput")
    tabw_d = nc.dram_tensor("tabw", [P, 8 * 2 * 32], FP8, kind="ExternalInput")
    tabc_d = nc.dram_tensor("tabc", [24, 6], BF16, kind="ExternalInput")
    tabio_d = nc.dram_tensor("tabio", [P, 64], I32, kind="ExternalInput")
    tabid_d = nc.dram_tensor("tabid", [6, 6], F32, kind="ExternalInput")
    out_d = nc.dram_tensor("out", [G, P, 256], I32, kind="ExternalOutput")

    with ExitStack() as ctx:
        sb = lambda name, shape, dt: ctx.enter_context(
            nc.sbuf_tensor(name, shape, dt))
        tabw_t = sb("tabw_t", [P, 8, 2, 32], FP8)
        tabc_t = sb("tabc_t", [24, 6], BF16)
        tabio_t = sb("tabio_t", [P, 8, 8], I32)   # word iota: [:, e, w] = w
        tabid_t = sb("tabid_t", [6, 6], F32)
        ones_t = sb("ones_t", [P, 4, 8], I32)
        in_t = [sb(f"in_t{k}", [P, 8, 2, NG], FP8) for k in range(NBUF)]
        nib = [sb(f"nib{k}", [24, NG], BF16) for k in range(3)]
        og = [sb(f"og{k}", [P, 4, 8, 8], I32) for k in range(OBUF)]
        eqt = [sb(f"eqt{k}", [P, 4, 8, 8], I32) for k in range(2)]
        actsc = sb("actsc", [P, 1], F32)
        # parity-double-buffered per-group temporaries (4 tiles x 8 lanes)
        iv = [sb(f"iv_{p}", [P, 4, 8], I32) for p in range(2)]
        idx8 = [sb(f"idx8_{p}", [P, 4, 8], I32) for p in range(2)]
        shv = [sb(f"shv_{p}", [P, 4, 8], I32) for p in range(2)]
        wiv = [sb(f"wiv_{p}", [P, 4, 8], I32) for p in range(2)]
        mv = [sb(f"mv_{p}", [P, 4, 8], I32) for p in range(2)]

        pn = [ctx.enter_context(nc.psum_tensor(f"pn{k}", [24, NG], F32))
              for k in range(2)]
        pt = [ctx.enter_context(nc.psum_tensor(f"pt{k}", [P, 4, 6], F32))
              for k in range(2)]

        s_tabw = ctx.enter_context(nc.semaphore("s_tabw"))
        s_tabc = ctx.enter_context(nc.semaphore("s_tabc"))
        s_tabid = ctx.enter_context(nc.semaphore("s_tabid"))
        s_tabio = ctx.enter_context(nc.semaphore("s_tabio"))
        s_in0 = [ctx.enter_context(nc.semaphore(f"s_in0_{u}"))
                 for u in range(NSUB)]
        s_in = [ctx.enter_context(nc.semaphore(f"s_in{j}"))
                for j in range(NBUF)]
        s_store = [ctx.enter_context(nc.semaphore(f"s_store{j}"))
                   for j in range(OBUF)]
        s_stl = ctx.enter_context(nc.semaphore("s_stl"))    # final store
        s_mm = ctx.enter_context(nc.semaphore("s_mm"))      # DoubleRow groups
        s_nb = ctx.enter_context(nc.semaphore("s_nb"))      # nib psum->sbuf
        s_T = ctx.enter_context(nc.semaphore("s_T"))        # transposes done
        s_cp = ctx.enter_context(nc.semaphore("s_cp"))      # ACT iv copies
        s_dve = ctx.enter_context(nc.semaphore("s_dve"))    # DVE op counter

        block = ctx.enter_context(nc.Block())

        @block.sync
        def _(sync: bass.BassEngine):
            NS2 = NSUB // 2   # group-0 sub-DMAs issued from sync

            # a tiny first DMA warms the queue + HBM path before the big
            # group-0 subs (half of which go on the ACT HWDGE ring, which
            # also carries tabw); the first DoubleRow matmul starts as
            # soon as sub 0 + tabw land
            sync.dma_start(out=tabio_t[:], in_=tabio_d[:]).then_inc(
                s_tabio, 16)
            for u in range(NS2):
                sync.dma_start(
                    out=in_t[0][:, 2 * u:2 * (u + 1)],
                    in_=ab_d[0, :, 2 * NG * 2 * u:2 * NG * 2 * (u + 1)],
                ).then_inc(s_in0[u], 16)
            # even groups on this ring; odd groups ride the ACT ring so
            # each in_t slot's semaphore is fed by exactly one HWDGE queue
            for g in range(2, G, 2):
                if g >= NBUF:
                    # slot reuse: matmuls of group g-NBUF consumed it
                    sync.wait_ge(s_mm, g - NBUF + 1)
                sync.dma_start(
                    out=in_t[g % NBUF][:], in_=ab_d[g],
                ).then_inc(s_in[g % NBUF], 16)

        @block.tensor
        def _(tensor: bass.BassEngine):
            tensor.wait_ge(s_tabw, 16)
            for g in range(G + 2):
                def combos(q):
                    # fused recombine+transpose: pt tile [128, 6] =
                    # nib[24, 128].T @ tabc[24, 6] -- replaces the pass2
                    # matmul, the sval PSUM->SBUF copy, AND the transposes
                    if q == 0:
                        tensor.wait_ge(s_tabc, 16)
                    tensor.wait_ge(s_nb, q + 1)
                    if q >= 2:
                        # pt[q%2] freed once ACT copied group q-2 to iv
                        tensor.wait_ge(s_cp, q - 1)
                    for k in range(4):
                        tensor.matmul(
                            out=pt[q % 2][:, k],
                            lhsT=nib[q % 3][:, P * k:P * (k + 1)],
                            rhs=tabc_t[:], start=True, stop=True,
                        ).then_inc(s_T, 1)

                # DoubleRow decode for group g: 8 matmuls, 2 K-chunks each
                if g < G:
                    j = g % NBUF
                    if g >= 2:
                        # pn[g%2] freed once ScalarE staged group g-2
                        tensor.wait_ge(s_nb, g - 1)
                    for cb in range(8):
                        if g == 0:
                            if cb % 2 == 0:
                                tensor.wait_ge(s_in0[cb // 2], 16)
                        elif cb == 0:
                            tensor.wait_ge(s_in[j], 16 * ((g - 1) // NBUF + 1)
                                           if j == 0 else 16 * (g // NBUF + 1))
                        ins = tensor.matmul(
                            out=pn[g % 2][:, :],
                            lhsT=tabw_t[:, cb, :, 0:24],
                            rhs=in_t[j][:, cb],
                            start=(cb == 0),
                            stop=(cb == 7),
                            perf_mode=mybir.MatmulPerfMode.DoubleRow,
                        )
                        if cb == 7:
                            ins.then_inc(s_mm, 1)
                if g == G - 1:
                    # tail: drain all remaining combos back-to-back right
                    # after the last DoubleRow group (the ACT nib copy of
                    # group G-1 lands during combos(G-3)/combos(G-2))
                    combos(G - 3)
                    combos(G - 2)
                    combos(G - 1)
                elif 0 <= g - 2 < G - 3:
                    combos(g - 2)

        @block.scalar
        def _(scalar: bass.BassEngine):
            NS2 = NSUB // 2
            # tabw + the other half of group 0 plus the small tables, on the
            # ACT HWDGE ring in parallel with sync's first subs
            scalar.dma_start(out=tabw_t[:], in_=tabw_d[:]).then_inc(
                s_tabw, 16)
            for u in range(NS2, NSUB):
                scalar.dma_start(
                    out=in_t[0][:, 2 * u:2 * (u + 1)],
                    in_=ab_d[0, :, 2048 * u:2048 * (u + 1)],
                ).then_inc(s_in0[u], 16)
            scalar.dma_start(out=tabid_t[:], in_=tabid_d[:]).then_inc(
                s_tabid, 16)
            scalar.dma_start(out=tabc_t[:], in_=tabc_d[:]).then_inc(
                s_tabc, 16)
            for g in (1, 3):
                scalar.dma_start(
                    out=in_t[g % NBUF][:], in_=ab_d[g],
                ).then_inc(s_in[g % NBUF], 16)
            # hoist the implicit ACT_TABLE_LOAD off the critical path
            scalar.wait_ge(s_tabw, 16)
            scalar.activation(
                out=actsc[:], in_=tabw_t[:, 0, 0, 0:1],
                func=mybir.ActivationFunctionType.Copy)
            for g in range(G + 2):
                # nib psum -> sbuf bf16 for group g
                if g < G:
                    scalar.wait_ge(s_mm, g + 1)
                    if g >= 3:
                        # nib[g%3] freed once combos of group g-3 done
                        scalar.wait_ge(s_T, 4 * (g - 2))
                    scalar.activation(
                        out=nib[g % 3][:, :], in_=pn[g % 2][:, :],
                        func=mybir.ActivationFunctionType.Copy,
                    ).then_inc(s_nb, 1)
                    # odd late-group loads: slot g+4 freed by DR(g) whose
                    # s_mm >= g+1 wait just passed
                    if g + 4 < G and (g + 4) % 2 == 1:
                        scalar.dma_start(
                            out=in_t[g % NBUF][:], in_=ab_d[g + 4],
                        ).then_inc(s_in[g % NBUF], 16)
                # pt -> iv int32 copy for group g-2 (one strided op: the
                # four [128, 6] tiles land at iv[:, k, 0:6])
                q = g - 2
                if 0 <= q < G:
                    scalar.wait_ge(s_T, 4 * (q + 1))
                    if q >= 2:
                        # iv[q%2] freed once DVE extB of group q-2 retired
                        scalar.wait_ge(s_dve, _cnt_chain(q - 2, 3))
                    scalar.activation(
                        out=iv[q % 2][:, :, 0:6],
                        in_=pt[q % 2][:],
                        func=mybir.ActivationFunctionType.Copy,
                    ).then_inc(s_cp, 1)
            # final store on the HWDGE ring (~1.4 us lower completion
            # latency than SWDGE; it ends the kernel)
            scalar.wait_ge(s_dve, _cnt_lastenc(G - 1))
            scalar.dma_start(
                out=out_d[G - 1], in_=og[(G - 1) % OBUF][:],
            ).then_inc(s_stl, 16)

        @block.gpsimd
        def _(gp: bass.BassEngine):
            # stores on the SWDGE queue keep the ACT ring free for loads;
            # the last store goes on the lower-latency ACT HWDGE ring
            for qs in range(G - 1):
                gp.wait_ge(s_dve, _cnt_lastenc(qs))
                gp.dma_start(
                    out=out_d[qs], in_=og[qs % OBUF][:],
                ).then_inc(s_store[qs % OBUF], 16)

        @block.vector
        def _(vector: bass.BassEngine):
            vector.wait_ge(s_tabio, 16)
            vector.memset(ones_t[:], 1).then_inc(s_dve, 1)

            def chain_ops(q):
                """7 group-wide chain ops for group q (list of closures)."""
                ivq = iv[q % 2]
                idxq = idx8[q % 2]
                ops = [
                    # x_lo/x_hi = a ^ b (16-bit halves; no cross-half carries)
                    lambda: vector.tensor_tensor(
                        out=ivq[:, :, 6:8], in0=ivq[:, :, 0:2],
                        in1=ivq[:, :, 2:4], op=mybir.AluOpType.bitwise_xor),
                    # fold the 2^16 carry into s_hi (s_lo keeps bit 16; the
                    # &255 byte masks strip it later)
                    lambda: vector.scalar_tensor_tensor(
                        out=ivq[:, :, 5:6], in0=ivq[:, :, 4:5], scalar=65536,
                        in1=ivq[:, :, 5:6],
                        op0=mybir.AluOpType.is_ge, op1=mybir.AluOpType.add),
                    # bytes, strided so they land [s0 s1 s2 s3 x0 x1 x2 x3]
                    lambda: vector.tensor_scalar(
                        out=idxq[:, :, 0:8:2], in0=ivq[:, :, 4:8],
                        scalar1=255, scalar2=None,
                        op0=mybir.AluOpType.bitwise_and),
                    lambda: vector.tensor_scalar(
                        out=idxq[:, :, 1:8:2], in0=ivq[:, :, 4:8],
                        scalar1=8, scalar2=255,
                        op0=mybir.AluOpType.logical_shift_right,
                        op1=mybir.AluOpType.bitwise_and),
                    # bit position, word index, bit mask
                    lambda: vector.tensor_scalar(
                        out=shv[q % 2][:], in0=idxq[:], scalar1=31,
                        scalar2=None, op0=mybir.AluOpType.bitwise_and),
                    lambda: vector.tensor_scalar(
                        out=wiv[q % 2][:], in0=idxq[:], scalar1=5,
                        scalar2=None,
                        op0=mybir.AluOpType.logical_shift_right),
                    lambda: vector.tensor_tensor(
                        out=mv[q % 2][:], in0=ones_t[:], in1=shv[q % 2][:],
                        op=mybir.AluOpType.logical_shift_left),
                ]
                return ops

            # chain op i -> index of its latest same-engine RAW producer
            CHAIN_WAIT = {2: 1, 3: 1, 4: 3, 5: 3, 6: 4}

            def enc_e1(q):
                par = q % 2
                return vector.tensor_tensor(
                    out=eqt[par][:],
                    in0=tabio_t[:, None, :, :].to_broadcast((P, 4, 8, 8)),
                    in1=wiv[par][:, :, :, None].to_broadcast((P, 4, 8, 8)),
                    op=mybir.AluOpType.is_equal)

            def enc_e2(q):
                par = q % 2
                return vector.tensor_tensor(
                    out=og[q % OBUF][:], in0=eqt[par][:],
                    in1=mv[par][:, :, :, None].to_broadcast((P, 4, 8, 8)),
                    op=mybir.AluOpType.mult)

            for q in range(G + 1):
                chain = chain_ops(q) if q < G else []
                qe = q - 1  # encode group
                if chain:
                    vector.wait_ge(s_cp, q + 1)
                if qe >= 0:
                    if qe >= OBUF:
                        # og slot reuse: store of group qe-OBUF drained
                        vector.wait_ge(s_store[qe % OBUF],
                                       16 * (qe // OBUF))
                    # wiv of group qe retired
                    vector.wait_ge(s_dve, _cnt_chain(qe, 5))

                def emit_chain(i):
                    if i in CHAIN_WAIT:
                        vector.wait_ge(s_dve, _cnt_chain(q, CHAIN_WAIT[i]))
                    chain[i]().then_inc(s_dve, 1)

                def emit_e1():
                    enc_e1(qe).then_inc(s_dve, 1)

                def emit_e2():
                    vector.wait_ge(s_dve, _cnt_e1(qe))
                    vector.wait_ge(s_dve, _cnt_chain(qe, 6))
                    enc_e2(qe).then_inc(s_dve, 1)

                if not chain:
                    # tail block: the two encode ops only
                    emit_e1()
                    emit_e2()
                    continue
                if qe < 0:
                    for i in range(7):
                        emit_chain(i)
                else:
                    # positions [c0 c1 E1 c2 c3 E2 c4 c5 c6]
                    emit_chain(0); emit_chain(1)
                    emit_e1()
                    emit_chain(2); emit_chain(3)
                    emit_e2()
                    emit_chain(4); emit_chain(5); emit_chain(6)

    return nc


def _make_tables():
    # DoubleRow decode weights: chunk c = 2*cb + i covers slab c//2
    # (a0..a3,b0..b3), half c%2; position pos = 128*(c%2) + k decomposes
    # into nibbles nL = pos & 15, nH = pos >> 4 -- all values in [0, 15],
    # exactly representable in fp8e4m3.
    k = np.arange(P)
    w2 = np.zeros((P, 8, 2, 32), np.float64)  # M padded to 32 (dual-fp8 ldweights needs 16B-aligned outer step)
    for cb in range(8):
        for i in range(2):
            c = 2 * cb + i
            slab, hh = c // 2, c % 2
            pos = 128 * hh + k
            nL, nH = pos & 15, pos >> 4
            j = slab % 4
            base = 0 if slab < 4 else 8
            w2[:, cb, i, base + 2 * j] = nL
            w2[:, cb, i, base + 2 * j + 1] = nH
            w2[:, cb, i, 16 + 2 * j] = nL
            w2[:, cb, i, 17 + 2 * j] = nH
    tabw = w2.reshape(P, 8 * 2 * 32).astype(ml_dtypes.float8_e4m3)
    # nibble -> half-sum recombination (16^j weights, bf16-exact)
    tabc = np.zeros((24, 6), np.float64)
    for col in range(6):
        for r in range(4):
            tabc[4 * col + r, col] = 16.0 ** r
    tabc = tabc.astype(ml_dtypes.bfloat16)
    tabio = np.tile(np.arange(8, dtype=np.int32), (P, 8, 1)).reshape(P, 64)
    tabio = np.ascontiguousarray(tabio)
    tabid = np.eye(6, dtype=np.float32)
    return tabw, tabc, tabio, tabid


def _pack_core(abt, lo):
    """[NCH, P, B] fp8 slab-chunks -> core block [G, P, NCH*NG]."""
    blk = abt[:, :, lo:lo + B_LOC].reshape(NCH, P, G, NG)
    return np.ascontiguousarray(
        blk.transpose(2, 1, 0, 3).reshape(G, P, NCH * NG))


def _unpack_out(res):
    """[G, P, 256] int32 bitmask block -> [B_LOC, 8, 256] uint8 rows."""
    bits = np.unpackbits(
        res.view(np.uint8).reshape(G, P, 4, 8, 32), axis=-1,
        bitorder="little")                      # [G, P, 4, 8, 256]
    return bits.transpose(0, 2, 1, 3, 4).reshape(B_LOC, 8, 256)


_NC_CACHE = {}


def _get_nc(variant: str = "main"):
    if variant not in _NC_CACHE:
        _NC_CACHE[variant] = _build_nc()
    return _NC_CACHE[variant]


def _run(a: np.ndarray, b: np.ndarray, **spmd_kwargs):
    assert a.shape == (B, 4, 256) and b.shape == (B, 4, 256)
    a_t = np.ascontiguousarray(
        np.asarray(a, np.float32).reshape(B, 4, 256).transpose(1, 2, 0)
    ).astype(ml_dtypes.float8_e4m3)
    b_t = np.ascontiguousarray(
        np.asarray(b, np.float32).reshape(B, 4, 256).transpose(1, 2, 0)
    ).astype(ml_dtypes.float8_e4m3)
    abt = np.concatenate([a_t.reshape(NCH // 2, P, B),
                          b_t.reshape(NCH // 2, P, B)], axis=0)
    tabw, tabc, tabio, tabid = _make_tables()
    in_maps = [
        {
            "abt": _pack_core(abt, i * B_LOC),
            "tabw": tabw,
            "tabc": tabc,
            "tabio": tabio,
            "tabid": tabid,
        }
        for i in range(N_CORES)
    ]
    nc = _get_nc()
    kr = run_bass_kernel_spmd(nc, in_maps, list(range(N_CORES)), **spmd_kwargs)
    out = np.empty((2, B, 4, 256), np.float32)
    for i in range(N_CORES):
        rows = _unpack_out(kr.results[i]["out"])  # [B_LOC, 8, 256] uint8
        lo = i * B_LOC
        out[0, lo:lo + B_LOC] = rows[:, 0:4]
        out[1, lo:lo + B_LOC] = rows[:, 4:8]
    return out, kr


def kernel(a: np.ndarray, b: np.ndarray) -> np.ndarray:
    out, _ = _run(a, b)
    return out


def run_sim():
    """CoreSim one core vs numpy oracle (invoked by test.py --sim)."""
    from concourse.bass_interp import CoreSim

    rng = np.random.default_rng(1)
    Bl = B_LOC
    ai = rng.integers(0, 256, (Bl, 4))
    bi = rng.integers(0, 256, (Bl, 4))
    ai[0] = [255] * 4
    bi[0] = [255] * 4
    ai[1] = [255, 255, 255, 255]
    bi[1] = [1, 0, 0, 0]
    a = np.zeros((Bl, 4, 256), np.float32)
    b = np.zeros((Bl, 4, 256), np.float32)
    r = np.arange(Bl)[:, None]
    j = np.arange(4)[None, :]
    a[r, j, ai] = 1.0
    b[r, j, bi] = 1.0

    a_t = np.ascontiguousarray(a.transpose(1, 2, 0)).astype(
        ml_dtypes.float8_e4m3)
    b_t = np.ascontiguousarray(b.transpose(1, 2, 0)).astype(
        ml_dtypes.float8_e4m3)
    abt = np.concatenate([a_t.reshape(NCH // 2, P, Bl),
                          b_t.reshape(NCH // 2, P, Bl)], axis=0)
    tabw, tabc, tabio, tabid = _make_tables()

    nc = _get_nc()
    sim = CoreSim(nc)
    sim.tensor("abt")[:] = _pack_core(abt, 0)
    sim.tensor("tabw")[:] = tabw
    sim.tensor("tabc")[:] = tabc
    sim.tensor("tabio")[:] = tabio
    sim.tensor("tabid")[:] = tabid
    sim.simulate()
    rows = _unpack_out(np.array(sim.tensor("out")))
    out = np.empty((2, Bl, 4, 256), np.float32)
    out[0] = rows[:, 0:4]
    out[1] = rows[:, 4:8]

    # numpy oracle
    pw = (256 ** np.arange(4)).astype(np.int64)
    a32 = (ai * pw).sum(-1)
    b32 = (bi * pw).sum(-1)
    s32 = (a32 + b32) % (2 ** 32)
    x32 = a32 ^ b32
    sb_ = np.stack([(s32 >> (8 * i)) & 255 for i in range(4)], -1)
    xb_ = np.stack([(x32 >> (8 * i)) & 255 for i in range(4)], -1)
    exp = np.zeros((2, Bl, 4, 256), np.float32)
    exp[0, r, j, sb_] = 1.0
    exp[1, r, j, xb_] = 1.0
    err = np.abs(out - exp).max()
    print(f"SIM max abs err: {err}")
    assert err == 0.0, "sim mismatch"
    print("SIM PASS")
